# revision 42
# baseline (speedup 1.0000x reference)
"""Trainium2 Bass kernel for nn_EnsembleSharedVQC: 12-qubit, 4-layer VQC ensemble
(4 encoders, shared theta), batch 1024, <Z_q> readout, softmax(alpha) mixture.

Sharding: pure data parallelism, 8 cores x 128 samples; theta/alpha replicated.
Everything is SBUF-resident per core (state = 128x4096 re/im f32 planes);
HBM traffic is just the tiny inputs/outputs.

Shipped pipeline (STAGE=2, TensorE-based, ~1.23 ms HW):
  - Encoding states are product states -> kron-doubling builds H (hi 7 qubits)
    and L (lo 5 qubits) factors with per-partition scalar multiplies;
    h_angle_rx encodes to |+>^12 up to a global phase -> constant state.
  - T-major layout S[h, b*32+u] with h = (q6, q0..q5) on partitions
    (q6 = MSB), u = (q7..q11) in the free dim.
  - Per layer l, on-device-built fused unitaries (by evolving the identity
    with the elementwise gate machinery, CNOTs folded):
      A_l = E_hi R_hi O_hi^{l-1}  (128x128 complex, hi qubits),
      B_l = E_lo R_lo O_lo^{l-1}  (32x32, replicated to blockdiag4).
    Layer = 4x {hi-MM chunks} -> 64x {PE transpose to U-major} ->
    {lo-MM with blockdiag(B) on q6=0 cols and blockdiag(X7 B) on q6=1 cols,
    consuming the crossing CNOT(6,7)} -> {PE transpose back}.
  - Measurement: only the last layer's odd CNOTs pend -> factorized hi/lo
    XOR-parities; signed marginals via sliced reduces + subtract-folds,
    per-sample outputs via one PE transpose per sign pattern.
  - softmax(alpha) mixing on device; out = [128, 12] f32 per core.

A stage-1 elementwise fallback (build_nc, ~6.15 ms: batch on partitions,
gates as DVE/ACT scalar_tensor_tensor passes, all CNOTs virtual via
reversed-dim reads) is kept for reference; STAGE selects the build.
"""

import numpy as np

import concourse.bass as bass
import concourse.bacc as bacc
import concourse.mybir as mybir
import concourse.tile as tile
from concourse.bass_utils import run_bass_kernel_spmd

AF = mybir.ActivationFunctionType
OP = mybir.AluOpType
F32 = mybir.dt.float32

N = 12
DIM = 1 << N
LAYERS = 4
B_CORE = 128
N_CORES = 8
ENCODERS = ["angle_rx", "angle_ry", "h_angle_rx", "h_angle_ry"]
INV_SQRT2 = float(1.0 / np.sqrt(2.0))
HALF_PI = float(np.pi / 2.0)

# physical-bit parity sets for <Z_q> under the final layer's pending flips
# (logical bit q = XOR of these physical bits; all runs are contiguous)
T_SETS = {0: [0], 1: [0, 1], 2: [0, 1, 2], 3: [2, 3], 4: [2, 3, 4], 5: [4, 5],
          6: [4, 5, 6], 7: [6, 7], 8: [6, 7, 8], 9: [8, 9], 10: [8, 9, 10],
          11: [10, 11]}


def _consume_flip(pend_list, q):
    """Remove pending flip with control q; return dims to reverse in s1 read."""
    idx = None
    for i, (c, _t) in enumerate(pend_list):
        if c == q:
            idx = i
            break
    if idx is None:
        return []
    mask = {pend_list[idx][1]}
    for c, t in reversed(pend_list[:idx]):
        if c in mask:
            mask.symmetric_difference_update({t})
    del pend_list[idx]
    return sorted(mask)


def _pieces(st_ap, q, a, rev_dims, flipped):
    """<=3D views of the (bit q == a) half of a [128, 4096] plane, split by the
    rev-dim combinations. flipped=True selects the bit-reversed sub-blocks (in
    the same positional order), used for reads that consume pending CNOTs."""
    L = 1 << q
    if not rev_dims:
        v = st_ap.rearrange("p (l a r) -> p l a r", l=L, a=2)
        return [v[:, :, a:a + 1, :]]
    if rev_dims == [q + 1]:
        w = st_ap.rearrange("p (l a t r) -> p l a t r", l=L, a=2, t=2)
        res = []
        for t in range(2):
            tt = 1 - t if flipped else t
            res.append(w[:, :, a:a + 1, tt:tt + 1, :])
        return res
    assert rev_dims == [q + 1, q + 2], rev_dims
    w = st_ap.rearrange("p (l a t u r) -> p l a t u r", l=L, a=2, t=2, u=2)
    res = []
    for t in range(2):
        for u in range(2):
            tt = 1 - t if flipped else t
            uu = 1 - u if flipped else u
            res.append(w[:, :, a:a + 1, tt:tt + 1, uu:uu + 1, :])
    return res


def build_nc():
    nc = bacc.Bacc(None, target_bir_lowering=False, debug=False)

    features = nc.dram_tensor("features", [B_CORE, N], F32, kind="ExternalInput").ap()
    theta = nc.dram_tensor("theta", [LAYERS, N, 3], F32, kind="ExternalInput").ap()
    alpha = nc.dram_tensor("alpha", [4], F32, kind="ExternalInput").ap()
    out = nc.dram_tensor("out", [B_CORE, N], F32, kind="ExternalOutput").ap()

    P = B_CORE

    with tile.TileContext(nc) as tc:
        with (
            tc.tile_pool(name="state", bufs=1) as state_pool,
            tc.tile_pool(name="small", bufs=1) as small,
            tc.tile_pool(name="scratch", bufs=4) as scratch,
        ):
            # ---------- input DMA ----------
            feat = small.tile([P, N], F32, tag="feat", name="feat")
            nc.sync.dma_start(out=feat, in_=features)

            th = small.tile([P, LAYERS, N, 3], F32, tag="th", name="th")
            th_b = bass.AP(tensor=theta.tensor, offset=0,
                           ap=[[0, P], [N * 3, LAYERS], [3, N], [1, 3]])
            nc.sync.dma_start(out=th, in_=th_b)

            alp = small.tile([P, 4], F32, tag="alp", name="alp")
            alp_b = bass.AP(tensor=alpha.tensor, offset=0, ap=[[0, P], [1, 4]])
            nc.sync.dma_start(out=alp, in_=alp_b)

            # ---------- trig of features ----------
            hpi = small.tile([P, 1], F32, tag="hpi", name="hpi")
            nc.vector.memset(hpi, HALF_PI)
            fh = small.tile([P, N], F32, tag="fh", name="fh")
            nc.scalar.activation(fh, feat, AF.Copy, scale=0.5)
            cf = small.tile([P, N], F32, tag="cf", name="cf")
            nc.scalar.activation(cf, fh, AF.Sin, bias=hpi)
            sf = small.tile([P, N], F32, tag="sf", name="sf")
            nc.scalar.activation(sf, fh, AF.Sin)
            nsf = small.tile([P, N], F32, tag="nsf", name="nsf")
            nc.vector.tensor_scalar_mul(nsf, sf, -1.0)
            # h_angle_ry vectors: hc = (c-s)/sqrt2, hs = (c+s)/sqrt2
            hc = small.tile([P, N], F32, tag="hc", name="hc")
            nc.vector.tensor_sub(hc, cf, sf)
            nc.vector.tensor_scalar_mul(hc, hc, INV_SQRT2)
            hs = small.tile([P, N], F32, tag="hs", name="hs")
            nc.vector.tensor_add(hs, cf, sf)
            nc.vector.tensor_scalar_mul(hs, hs, INV_SQRT2)

            # ---------- trig of theta & fused gate coefficients ----------
            def flat(ap):
                return ap.rearrange("p a b c -> p (a b c)")

            thh = small.tile([P, LAYERS, N, 3], F32, tag="thh", name="thh")
            nc.scalar.activation(flat(thh), flat(th), AF.Copy, scale=0.5)
            ct = small.tile([P, LAYERS, N, 3], F32, tag="ct", name="ct")
            nc.scalar.activation(flat(ct), flat(thh), AF.Sin, bias=hpi)
            st = small.tile([P, LAYERS, N, 3], F32, tag="st", name="st")
            nc.scalar.activation(flat(st), flat(thh), AF.Sin)

            ca, cb, cg = (ct[:, :, :, i:i + 1] for i in range(3))
            sa, sb, sg = (st[:, :, :, i:i + 1] for i in range(3))

            def lq_tile(tag):
                return small.tile([P, LAYERS, N, 1], F32, tag=tag, name=tag)

            t1, t2, t3, t4 = (lq_tile(f"t{i}") for i in range(4))
            nc.vector.tensor_mul(t1, cg, cb)
            nc.vector.tensor_mul(t2, sg, sb)
            nc.vector.tensor_mul(t3, cg, sb)
            nc.vector.tensor_mul(t4, sg, cb)

            u1, u2 = lq_tile("u1"), lq_tile("u2")
            w_c, x_c, y_c, z_c = (lq_tile(t) for t in ("w", "x", "y", "z"))
            nx_c, ny_c, nz_c = (lq_tile(t) for t in ("nx", "ny", "nz"))
            # w = t1*ca + t2*sa
            nc.vector.tensor_mul(u1, t1, ca)
            nc.vector.tensor_mul(u2, t2, sa)
            nc.vector.tensor_add(w_c, u1, u2)
            # x = t3*sa - t4*ca
            nc.vector.tensor_mul(u1, t3, sa)
            nc.vector.tensor_mul(u2, t4, ca)
            nc.vector.tensor_sub(x_c, u1, u2)
            nc.vector.tensor_scalar_mul(nx_c, x_c, -1.0)
            # ny = t3*ca + t4*sa ; y = -ny
            nc.vector.tensor_mul(u1, t3, ca)
            nc.vector.tensor_mul(u2, t4, sa)
            nc.vector.tensor_add(ny_c, u1, u2)
            nc.vector.tensor_scalar_mul(y_c, ny_c, -1.0)
            # z = t2*ca - t1*sa
            nc.vector.tensor_mul(u1, t2, ca)
            nc.vector.tensor_mul(u2, t1, sa)
            nc.vector.tensor_sub(z_c, u1, u2)
            nc.vector.tensor_scalar_mul(nz_c, z_c, -1.0)

            def coef(c, l, q):
                return c[:, l:l + 1, q:q + 1, :]

            # ---------- softmax(alpha) ----------
            amax = small.tile([P, 1], F32, tag="amax", name="amax")
            nc.vector.reduce_max(amax, alp, axis=mybir.AxisListType.X)
            esh = small.tile([P, 4], F32, tag="esh", name="esh")
            nc.vector.tensor_scalar(esh, alp, amax, None, op0=OP.subtract)
            nc.scalar.activation(esh, esh, AF.Exp)
            ssum = small.tile([P, 1], F32, tag="ssum", name="ssum")
            nc.vector.reduce_sum(ssum, esh, axis=mybir.AxisListType.X)
            rsum = small.tile([P, 1], F32, tag="rsum", name="rsum")
            nc.vector.reciprocal(rsum, ssum)
            wts = small.tile([P, 4], F32, tag="wts", name="wts")
            nc.vector.tensor_scalar(wts, esh, rsum, None, op0=OP.mult)

            # ---------- state buffers ----------
            re_a = state_pool.tile([P, DIM], F32, tag="re_a", name="re_a")
            im_a = state_pool.tile([P, DIM], F32, tag="im_a", name="im_a")
            re_b = state_pool.tile([P, DIM], F32, tag="re_b", name="re_b")
            im_b = state_pool.tile([P, DIM], F32, tag="im_b", name="im_b")

            zacc = small.tile([P, N], F32, tag="zacc", name="zacc")
            nc.vector.memset(zacc, 0.0)

            for enc_i, enc in enumerate(ENCODERS):
                # ---------- encoding: build product state in (re_a, im_a) ----------
                if enc == "h_angle_rx":
                    nc.vector.memset(re_a, float(2.0 ** -6))
                    nc.gpsimd.memset(im_a, 0.0)
                else:
                    nc.gpsimd.memset(im_a, 0.0)
                    nc.vector.memset(re_a[:, 0:1], 1.0)
                    size = 1
                    for q in range(N - 1, -1, -1):
                        lo = re_a[:, 0:size]
                        loi = im_a[:, 0:size]
                        hi = re_a[:, size:2 * size]
                        hii = im_a[:, size:2 * size]
                        if enc == "angle_rx":
                            # v0 = (c, 0); v1 = (0, -s)
                            v0 = cf[:, q:q + 1]
                            nc.scalar.mul(hi, loi, sf[:, q:q + 1])
                            nc.scalar.mul(hii, lo, nsf[:, q:q + 1])
                            nc.scalar.mul(loi, loi, v0)
                            nc.scalar.mul(lo, lo, v0)
                        else:
                            # real vectors: v0 = (a, 0); v1 = (b, 0); im stays 0
                            if enc == "angle_ry":
                                a_ap, b_ap = cf[:, q:q + 1], sf[:, q:q + 1]
                            else:  # h_angle_ry
                                a_ap, b_ap = hc[:, q:q + 1], hs[:, q:q + 1]
                            nc.scalar.mul(hi, lo, b_ap)
                            nc.scalar.mul(lo, lo, a_ap)
                        size *= 2

                # ---------- variational layers ----------
                pend = []
                cur = (re_a, im_a)
                nxt = (re_b, im_b)
                for l in range(LAYERS):
                    for q in range(N):
                        rev = _consume_flip(pend, q)
                        pr, pi = cur
                        qr, qi = nxt
                        pr0 = _pieces(pr, q, 0, [], False)[0]
                        pi0 = _pieces(pi, q, 0, [], False)[0]
                        pr1p = _pieces(pr, q, 1, rev, True)
                        pi1p = _pieces(pi, q, 1, rev, True)
                        w = coef(w_c, l, q)
                        x = coef(x_c, l, q)
                        y = coef(y_c, l, q)
                        z = coef(z_c, l, q)
                        nx = coef(nx_c, l, q)
                        ny = coef(ny_c, l, q)
                        nz = coef(nz_c, l, q)
                        stt = nc.vector.scalar_tensor_tensor
                        # (out_plane, a, c_re0, c_im0, c_re1, c_im1):
                        #  out = c_re0*pr0 + c_im0*pi0 + c_re1*pr1 + c_im1*pi1
                        for out_pl, a, c1, c2, c3, c4 in (
                            (qr, 0, w, nx, y, nz),   # re_out0
                            (qi, 0, x, w, z, y),     # im_out0
                            (qr, 1, ny, nz, w, x),   # re_out1
                            (qi, 1, z, ny, nx, w),   # im_out1
                        ):
                            o_full = _pieces(out_pl, q, a, [], False)[0]
                            o_sub = _pieces(out_pl, q, a, rev, False)
                            nc.scalar.mul(o_full, pr0, c1)
                            stt(o_full, pi0, c2, o_full,
                                op0=OP.mult, op1=OP.add)
                            for o_p, i_p in zip(o_sub, pr1p):
                                stt(o_p, i_p, c3, o_p, op0=OP.mult, op1=OP.add)
                            for o_p, i_p in zip(o_sub, pi1p):
                                stt(o_p, i_p, c4, o_p, op0=OP.mult, op1=OP.add)
                        cur, nxt = nxt, cur
                    assert not pend
                    pend = [(q, q + 1) for q in range(0, N - 1, 2)] + \
                           [(q, q + 1) for q in range(1, N - 1, 2)]

                # after an even number of gates, state is back in (re_a, im_a)
                fr, fi = cur

                # ---------- measurement ----------
                p = nxt[0]  # reuse a pong plane for |psi|^2
                sq = nxt[1]
                nc.scalar.activation(p, fr, AF.Square)
                nc.scalar.activation(sq, fi, AF.Square)
                nc.vector.tensor_add(p, p, sq)

                z_e = small.tile([P, N], F32, tag=f"z_e{enc_i}", name=f"z_e{enc_i}")

                for q in range(N):
                    T = T_SETS[q]
                    a, b = T[0], T[-1]
                    nT = b - a + 1
                    outer = 1 << a
                    run = 1 << nT
                    inner = DIM // (outer * run)
                    # marginal over non-T bits (two <=3D reduce stages)
                    if inner > 1:
                        r1 = scratch.tile([P, outer * run], F32,
                                          tag="marg1", name="marg1")
                        nc.vector.reduce_sum(
                            r1, p.rearrange("p (a i) -> p a i", i=inner),
                            axis=mybir.AxisListType.X)
                    else:
                        r1 = p
                    if outer > 1:
                        m = scratch.tile([P, run], F32, tag="marg", name="marg")
                        nc.vector.reduce_sum(
                            m, r1.rearrange("p (o t) -> p t o", o=outer),
                            axis=mybir.AxisListType.X)
                    else:
                        m = r1
                    # subtract-fold nT times
                    src = m
                    width = run
                    for _k in range(nT):
                        width //= 2
                        dst = (z_e[:, q:q + 1] if width == 1
                               else scratch.tile([P, width], F32, tag="fold", name="fold"))
                        nc.vector.tensor_sub(dst, src[:, 0:width],
                                             src[:, width:2 * width])
                        src = dst

                # weighted accumulate
                nc.vector.scalar_tensor_tensor(
                    zacc, z_e, wts[:, enc_i:enc_i + 1], zacc,
                    op0=OP.mult, op1=OP.add)

            # ---------- output ----------
            nc.sync.dma_start(out=out, in_=zacc)

    nc.finalize()
    return nc


# ======================================================================
# Stage 2: TensorE-based pipeline.
#   T-major: S[h, b*32+u], h = (q6, q0..q5) bits (q6 = MSB),
#            u = (q7..q11) bits, b = sample.
#   Per layer: hi-MM (A_l = E_hi R_hi O_hi^{l-1}, 128x128 complex);
#   PE-transpose to U-major [(b%4,u), q6*2048 + (b//4)*64 + h'];
#   lo-MM with blockdiag4(B_l) on q6=0 cols / blockdiag4(X7 B_l) on q6=1
#   cols (consumes crossing CNOT(6,7)); PE-transpose back.
#   Measurement: pending O-flips only -> factorized hi/lo parities.
# ======================================================================

HI_ORDER = [6, 0, 1, 2, 3, 4, 5]
LO_ORDER = [7, 8, 9, 10, 11]
E_HI = [(0, 1), (2, 3), (4, 5)]
O_HI = [(1, 2), (3, 4), (5, 6)]
E_LO = [(8, 9), (10, 11)]
O_LO = [(7, 8), (9, 10)]

MM_DTYPE = F32  # float32r would need fp32r-rounded producers throughout


def _bitview(ap, nbits, fixed):
    """View a [P, 2**nbits] plane with some bit positions fixed.

    Returns an AP [P, 2,2,...] sliced at the fixed positions (count-1 dims);
    opt-merging at lowering keeps it <=3 free dims for <=2 fixed bits."""
    names = [f"b{i}" for i in range(nbits)]
    pat = "p ({}) -> p {}".format(" ".join(names), " ".join(names))
    v = ap.rearrange(pat, **{n: 2 for n in names[:-1]})
    idx = [slice(None)] * (nbits + 1)
    for pos, val in fixed.items():
        idx[1 + pos] = slice(val, val + 1)
    return v[tuple(idx)]


def _small_gate(nc, cur, nxt, nbits, pos, rev, cw, npart):
    """Fused SU(2) gate on wire `pos` of an npart x 2**nbits state.

    cur/nxt: (re_ap, im_ap) ping/pong, full [128, 2**nbits] tiles; only
    partitions [0:npart] are used. rev: list of wire positions reversed in the
    s1 read (consumed CNOTs). cw: dict with w,x,y,z,nx,ny,nz [P,1] scalar APs."""
    pr, pi = cur
    qr, qi = nxt
    cw = {k: v[:npart] for k, v in cw.items()}
    combos = [()]
    for _ in rev:
        combos = [c + (v,) for c in combos for v in (0, 1)]
    pr0 = _bitview(pr, nbits, {pos: 0})[:npart]
    pi0 = _bitview(pi, nbits, {pos: 0})[:npart]
    stt = nc.vector.scalar_tensor_tensor
    for out_pl, a, c1, c2, c3, c4 in (
        (qr, 0, cw['w'], cw['nx'], cw['y'], cw['nz']),
        (qi, 0, cw['x'], cw['w'], cw['z'], cw['y']),
        (qr, 1, cw['ny'], cw['nz'], cw['w'], cw['x']),
        (qi, 1, cw['z'], cw['ny'], cw['nx'], cw['w']),
    ):
        o_full = _bitview(out_pl, nbits, {pos: a})[:npart]
        nc.scalar.mul(o_full, pr0, c1)
        stt(o_full, pi0, c2, o_full, op0=OP.mult, op1=OP.add)
        for combo in combos:
            ofix = {pos: a}
            ifix = {pos: 1}
            for rp, v in zip(rev, combo):
                ofix[rp] = v
                ifix[rp] = 1 - v
            o_p = _bitview(out_pl, nbits, ofix)[:npart]
            pr1 = _bitview(pr, nbits, ifix)[:npart]
            pi1 = _bitview(pi, nbits, ifix)[:npart]
            stt(o_p, pr1, c3, o_p, op0=OP.mult, op1=OP.add)
            stt(o_p, pi1, c4, o_p, op0=OP.mult, op1=OP.add)


def _small_swap(nc, planes, nbits, c_pos, t_pos, tmp, npart):
    """Physical CNOT(c_pos -> t_pos) swap on an npart x 2**nbits state."""
    qdim = 1 << (nbits - 2)
    for pl in planes:
        v0 = _bitview(pl, nbits, {c_pos: 1, t_pos: 0})[:npart]
        v1 = _bitview(pl, nbits, {c_pos: 1, t_pos: 1})[:npart]
        t = tmp[:npart, 0:qdim]
        nc.vector.tensor_copy(t, v0)
        nc.scalar.copy(v0, v1)
        nc.vector.tensor_copy(v1, t)


def build_nc_stage2(trunc=None):
    """trunc: None = full kernel. Otherwise (enc_count, layer_count, level):
    emit only `enc_count` encoders, `layer_count` layers, and within the last
    layer stop after `level` in {hi, tr, lo, trb, full}; adds a dbg output
    dumping (S_re | U_re | V_re) for comparison against the numpy plan."""
    nc = bacc.Bacc(None, target_bir_lowering=False, debug=False)

    features = nc.dram_tensor("features", [B_CORE, N], F32, kind="ExternalInput").ap()
    theta = nc.dram_tensor("theta", [LAYERS, N, 3], F32, kind="ExternalInput").ap()
    alpha = nc.dram_tensor("alpha", [4], F32, kind="ExternalInput").ap()
    out = nc.dram_tensor("out", [B_CORE, N], F32, kind="ExternalOutput").ap()
    dbg = None
    if trunc is not None:
        dbg = nc.dram_tensor("dbg", [B_CORE, 2 * DIM], F32,
                             kind="ExternalOutput").ap()

    P = B_CORE
    AX = mybir.AxisListType

    from concourse.masks import make_identity

    with tile.TileContext(nc) as tc:
        with (
            tc.tile_pool(name="state", bufs=1) as sp,
            tc.tile_pool(name="small", bufs=1) as small,
            tc.tile_pool(name="scratch", bufs=4) as scratch,
            tc.tile_pool(name="mm", bufs=4, space="PSUM") as mmpool,
            tc.tile_pool(name="tp", bufs=4, space="PSUM") as tppool,
            tc.tile_pool(name="dram", bufs=2, space="DRAM") as dpool,
        ):
            # ---------- input DMA ----------
            feat = small.tile([P, N], F32, tag="feat", name="feat")
            nc.sync.dma_start(out=feat, in_=features)
            th = small.tile([P, LAYERS, N, 3], F32, tag="th", name="th")
            th_b = bass.AP(tensor=theta.tensor, offset=0,
                           ap=[[0, P], [N * 3, LAYERS], [3, N], [1, 3]])
            nc.sync.dma_start(out=th, in_=th_b)
            alp = small.tile([P, 4], F32, tag="alp", name="alp")
            alp_b = bass.AP(tensor=alpha.tensor, offset=0, ap=[[0, P], [1, 4]])
            nc.sync.dma_start(out=alp, in_=alp_b)

            # ---------- trig ----------
            hpi = small.tile([P, 1], F32, tag="hpi", name="hpi")
            nc.vector.memset(hpi, HALF_PI)
            fh = small.tile([P, N], F32, tag="fh", name="fh")
            nc.scalar.activation(fh, feat, AF.Copy, scale=0.5)
            cf = small.tile([P, N], F32, tag="cf", name="cf")
            nc.scalar.activation(cf, fh, AF.Sin, bias=hpi)
            sf = small.tile([P, N], F32, tag="sf", name="sf")
            nc.scalar.activation(sf, fh, AF.Sin)
            nsf = small.tile([P, N], F32, tag="nsf", name="nsf")
            nc.vector.tensor_scalar_mul(nsf, sf, -1.0)
            hc = small.tile([P, N], F32, tag="hc", name="hc")
            nc.vector.tensor_sub(hc, cf, sf)
            nc.vector.tensor_scalar_mul(hc, hc, INV_SQRT2)
            hs = small.tile([P, N], F32, tag="hs", name="hs")
            nc.vector.tensor_add(hs, cf, sf)
            nc.vector.tensor_scalar_mul(hs, hs, INV_SQRT2)

            def flat(ap):
                return ap.rearrange("p a b c -> p (a b c)")

            thh = small.tile([P, LAYERS, N, 3], F32, tag="thh", name="thh")
            nc.scalar.activation(flat(thh), flat(th), AF.Copy, scale=0.5)
            ct = small.tile([P, LAYERS, N, 3], F32, tag="ct", name="ct")
            nc.scalar.activation(flat(ct), flat(thh), AF.Sin, bias=hpi)
            st = small.tile([P, LAYERS, N, 3], F32, tag="st", name="st")
            nc.scalar.activation(flat(st), flat(thh), AF.Sin)

            ca, cb, cg = (ct[:, :, :, i:i + 1] for i in range(3))
            sa, sb, sg = (st[:, :, :, i:i + 1] for i in range(3))

            def lq_tile(tag):
                return small.tile([P, LAYERS, N, 1], F32, tag=tag, name=tag)

            t1, t2, t3, t4 = (lq_tile(f"t{i}") for i in range(4))
            nc.vector.tensor_mul(t1, cg, cb)
            nc.vector.tensor_mul(t2, sg, sb)
            nc.vector.tensor_mul(t3, cg, sb)
            nc.vector.tensor_mul(t4, sg, cb)
            u1, u2 = lq_tile("u1"), lq_tile("u2")
            w_c, x_c, y_c, z_c = (lq_tile(t) for t in ("w", "x", "y", "z"))
            nx_c, ny_c, nz_c = (lq_tile(t) for t in ("nx", "ny", "nz"))
            nc.vector.tensor_mul(u1, t1, ca)
            nc.vector.tensor_mul(u2, t2, sa)
            nc.vector.tensor_add(w_c, u1, u2)
            nc.vector.tensor_mul(u1, t3, sa)
            nc.vector.tensor_mul(u2, t4, ca)
            nc.vector.tensor_sub(x_c, u1, u2)
            nc.vector.tensor_scalar_mul(nx_c, x_c, -1.0)
            nc.vector.tensor_mul(u1, t3, ca)
            nc.vector.tensor_mul(u2, t4, sa)
            nc.vector.tensor_add(ny_c, u1, u2)
            nc.vector.tensor_scalar_mul(y_c, ny_c, -1.0)
            nc.vector.tensor_mul(u1, t2, ca)
            nc.vector.tensor_mul(u2, t1, sa)
            nc.vector.tensor_sub(z_c, u1, u2)
            nc.vector.tensor_scalar_mul(nz_c, z_c, -1.0)

            def coefs(l, q):
                return {k: c[:, l:l + 1, q:q + 1, :] for k, c in
                        (('w', w_c), ('x', x_c), ('y', y_c), ('z', z_c),
                         ('nx', nx_c), ('ny', ny_c), ('nz', nz_c))}

            # ---------- softmax(alpha) ----------
            amax = small.tile([P, 1], F32, tag="amax", name="amax")
            nc.vector.reduce_max(amax, alp, axis=AX.X)
            esh = small.tile([P, 4], F32, tag="esh", name="esh")
            nc.vector.tensor_scalar(esh, alp, amax, None, op0=OP.subtract)
            nc.scalar.activation(esh, esh, AF.Exp)
            ssum = small.tile([P, 1], F32, tag="ssum", name="ssum")
            nc.vector.reduce_sum(ssum, esh, axis=AX.X)
            rsum = small.tile([P, 1], F32, tag="rsum", name="rsum")
            nc.vector.reciprocal(rsum, ssum)
            wts = small.tile([P, 4], F32, tag="wts", name="wts")
            nc.vector.tensor_scalar(wts, esh, rsum, None, op0=OP.mult)

            # ---------- identity for PE transposes ----------
            ident = small.tile([P, P], F32, tag="ident", name="ident")
            make_identity(nc, ident)

            def mmcast(ap):
                return ap.bitcast(MM_DTYPE) if MM_DTYPE != F32 else ap

            # ---------- build A_l (hi) and B_l (lo) stationaries ----------
            # build scratch (ping/pong pairs)
            bre_a = small.tile([P, P], F32, tag="bre_a", name="bre_a")
            bim_a = small.tile([P, P], F32, tag="bim_a", name="bim_a")
            bre_b = small.tile([P, P], F32, tag="bre_b", name="bre_b")
            bim_b = small.tile([P, P], F32, tag="bim_b", name="bim_b")
            btmp = small.tile([P, 64], F32, tag="btmp", name="btmp")

            A_t = []   # per layer: (Ar, Ai, nAi) [128,128] (lhsT = A^T)
            B_t = []   # per layer: (Br, Bi, nBi, BrX, BiX, nBiX) blockdiag lhsT

            def build_small(order, qubits, flips_pre, flips_post, l, npart):
                nbits = {7: 7, 5: 5}[len(order)]
                dim = 1 << nbits
                make_identity(nc, bre_a[:dim, :dim])
                nc.gpsimd.memset(bim_a[:, 0:dim], 0.0)
                cur = (bre_a[:, 0:dim], bim_a[:, 0:dim])
                nxt = (bre_b[:, 0:dim], bim_b[:, 0:dim])
                pend = [(order.index(c), order.index(t)) for c, t in flips_pre]
                for q in qubits:
                    pos = order.index(q)
                    rev = [t for c_, t in pend if c_ == pos]
                    pend = [(c_, t) for c_, t in pend if c_ != pos]
                    _small_gate(nc, cur, nxt, nbits, pos, rev, coefs(l, q), npart)
                    cur, nxt = nxt, cur
                assert not pend
                for c_, t in flips_post:
                    _small_swap(nc, cur, nbits, order.index(c_),
                                order.index(t), btmp, npart)
                return cur

            for l in range(LAYERS):
                ohi = O_HI if l > 0 else []
                olo = O_LO if l > 0 else []
                curA = build_small(HI_ORDER, [0, 1, 2, 3, 4, 5, 6], ohi, E_HI, l, P)
                Ar = small.tile([P, P], F32, tag=f"Ar{l}", name=f"Ar{l}")
                Ai = small.tile([P, P], F32, tag=f"Ai{l}", name=f"Ai{l}")
                nAi = small.tile([P, P], F32, tag=f"nAi{l}", name=f"nAi{l}")
                nc.vector.tensor_copy(Ar, curA[0])
                nc.vector.tensor_copy(Ai, curA[1])
                nc.vector.tensor_scalar_mul(nAi, curA[1], -1.0)
                A_t.append((Ar, Ai, nAi))

                curB = build_small(LO_ORDER, [7, 8, 9, 10, 11], olo, E_LO, l, 32)
                # blockdiag4 of B^T (and of (X7 B)^T = B^T with free halves
                # swapped), + negated-imag variants
                names = (f"Br{l}", f"Bi{l}", f"nBi{l}",
                         f"BrX{l}", f"BiX{l}", f"nBiX{l}")
                tiles = []
                for nm in names:
                    tt = small.tile([P, P], F32, tag=nm, name=nm)
                    nc.gpsimd.memset(tt, 0.0)
                    tiles.append(tt)
                Br, Bi, nBi, BrX, BiX, nBiX = tiles
                negB = small.tile([P, 32], F32, tag="negB", name="negB")
                nc.vector.tensor_scalar_mul(negB[:32], curB[1][:32, :32], -1.0)
                for i in range(4):
                    sl = slice(32 * i, 32 * i + 32)
                    for dst, src in ((Br, curB[0][:32, :32]),
                                     (Bi, curB[1][:32, :32]),
                                     (nBi, negB[:32])):
                        nc.sync.dma_start(out=dst[sl, sl], in_=src)
                    # X-variant: free halves swapped (col m -> m^16)
                    for dst, src in ((BrX, curB[0][:32, :32]),
                                     (BiX, curB[1][:32, :32]),
                                     (nBiX, negB[:32])):
                        swp = dst[sl, sl].rearrange("p (t r) -> p t r", t=2)
                        nc.sync.dma_start(
                            out=swp, in_=src.rearrange(
                                "p (t r) -> p t r", t=2)[:, ::-1, :])
                B_t.append((Br, Bi, nBi, BrX, BiX, nBiX))

            # ---------- state planes ----------
            S_re = sp.tile([P, DIM], F32, tag="S_re", name="S_re")
            S_im = sp.tile([P, DIM], F32, tag="S_im", name="S_im")
            T_re = sp.tile([P, DIM], F32, tag="T_re", name="T_re")
            T_im = sp.tile([P, DIM], F32, tag="T_im", name="T_im")
            U_re = sp.tile([P, DIM], F32, tag="U_re", name="U_re")
            U_im = sp.tile([P, DIM], F32, tag="U_im", name="U_im")
            V_re = sp.tile([P, DIM], F32, tag="V_re", name="V_re")
            V_im = sp.tile([P, DIM], F32, tag="V_im", name="V_im")

            # encoding scratch
            Hb_re = small.tile([P, P], F32, tag="Hb_re", name="Hb_re")
            Hb_im = small.tile([P, P], F32, tag="Hb_im", name="Hb_im")
            H_re = small.tile([P, P], F32, tag="H_re", name="H_re")
            H_im = small.tile([P, P], F32, tag="H_im", name="H_im")
            Lb_re = small.tile([P, 32], F32, tag="Lb_re", name="Lb_re")
            Lb_im = small.tile([P, 32], F32, tag="Lb_im", name="Lb_im")

            zacc = small.tile([P, N], F32, tag="zacc", name="zacc")
            nc.vector.memset(zacc, 0.0)

            def doubling(re_t, im_t, order, enc, cplx):
                """Kron-doubling over `order` wires into [P, 2**n] tiles
                (b on partitions)."""
                nc.vector.memset(re_t[:, 0:1], 1.0)
                if cplx:
                    nc.gpsimd.memset(im_t, 0.0)
                size = 1
                for q in reversed(order):
                    lo = re_t[:, 0:size]
                    hi = re_t[:, size:2 * size]
                    if enc == "angle_rx":
                        loi = im_t[:, 0:size]
                        hii = im_t[:, size:2 * size]
                        v0 = cf[:, q:q + 1]
                        nc.scalar.mul(hi, loi, sf[:, q:q + 1])
                        nc.scalar.mul(hii, lo, nsf[:, q:q + 1])
                        nc.scalar.mul(loi, loi, v0)
                        nc.scalar.mul(lo, lo, v0)
                    else:
                        if enc == "angle_ry":
                            a_ap, b_ap = cf[:, q:q + 1], sf[:, q:q + 1]
                        else:
                            a_ap, b_ap = hc[:, q:q + 1], hs[:, q:q + 1]
                        nc.scalar.mul(hi, lo, b_ap)
                        nc.scalar.mul(lo, lo, a_ap)
                    size *= 2

            def u_major_view(plane, g):
                """[p, 2(q6), 64(h')] view of U-major cols for block g."""
                v = plane.rearrange("p (s g2 h) -> p s g2 h", s=2, g2=32)
                return v[:, :, g:g + 1, :]

            encs = ENCODERS[:trunc[0]] if trunc else ENCODERS
            nlayers = trunc[1] if trunc else LAYERS
            tlevel = trunc[2] if trunc else 'full'
            dbg_src = [S_re, S_im]

            for enc_i, enc in enumerate(encs):
                # ---------- encoding ----------
                if enc == "h_angle_rx":
                    nc.vector.memset(S_re, float(2.0 ** -6))
                    nc.gpsimd.memset(S_im, 0.0)
                else:
                    cplx = enc == "angle_rx"
                    doubling(Hb_re, Hb_im, HI_ORDER, enc, cplx)
                    doubling(Lb_re, Lb_im, LO_ORDER, enc, cplx)
                    # transpose Hb -> H
                    pt = tppool.tile([P, P], F32, tag="tp", name="ptH")
                    nc.tensor.transpose(pt, Hb_re, ident)
                    nc.scalar.copy(H_re, pt)
                    if cplx:
                        pt2 = tppool.tile([P, P], F32, tag="tp", name="ptH2")
                        nc.tensor.transpose(pt2, Hb_im, ident)
                        nc.scalar.copy(H_im, pt2)
                    # broadcast L over partitions via DRAM roundtrip
                    dl_re = dpool.tile([P, 32], F32, tag="dl_re", name="dl_re")
                    nc.sync.dma_start(out=dl_re, in_=Lb_re)
                    lrow_re = bass.AP(tensor=dl_re.tensor, offset=dl_re.offset,
                                      ap=[[0, P], [1, DIM]])
                    nc.sync.dma_start(out=V_re, in_=lrow_re)
                    if cplx:
                        dl_im = dpool.tile([P, 32], F32, tag="dl_im",
                                           name="dl_im")
                        nc.sync.dma_start(out=dl_im, in_=Lb_im)
                        lrow_im = bass.AP(tensor=dl_im.tensor,
                                          offset=dl_im.offset,
                                          ap=[[0, P], [1, DIM]])
                        nc.sync.dma_start(out=V_im, in_=lrow_im)
                    # S = (H bcast over u) * L_bc  (complex)
                    def hview(hp):
                        return hp.unsqueeze(2).broadcast_to((P, P, 32))
                    sv_re = S_re.rearrange("p (b u) -> p b u", u=32)
                    sv_im = S_im.rearrange("p (b u) -> p b u", u=32)
                    lv_re = V_re.rearrange("p (b u) -> p b u", u=32)
                    if cplx:
                        lv_im = V_im.rearrange("p (b u) -> p b u", u=32)
                        tv_re = T_re.rearrange("p (b u) -> p b u", u=32)
                        nc.vector.tensor_tensor(sv_re, hview(H_re), lv_re,
                                                op=OP.mult)
                        nc.vector.tensor_tensor(tv_re, hview(H_im), lv_im,
                                                op=OP.mult)
                        nc.vector.tensor_sub(sv_re, sv_re, tv_re)
                        nc.vector.tensor_tensor(sv_im, hview(H_re), lv_im,
                                                op=OP.mult)
                        nc.vector.tensor_tensor(tv_re, hview(H_im), lv_re,
                                                op=OP.mult)
                        nc.vector.tensor_add(sv_im, sv_im, tv_re)
                    else:
                        nc.vector.tensor_tensor(sv_re, hview(H_re), lv_re,
                                                op=OP.mult)
                        nc.gpsimd.memset(S_im, 0.0)

                # ---------- layers ----------
                # h_angle_rx is sample-independent: evolve only block g=0
                # (samples 0-3) and broadcast afterwards.
                fast3 = enc == "h_angle_rx" and trunc is None
                for l in range(nlayers):
                    last = trunc is not None and l == nlayers - 1
                    if last and tlevel == 'enc':
                        break
                    Ar, Ai, nAi = A_t[l]
                    Br, Bi, nBi, BrX, BiX, nBiX = B_t[l]
                    if fast3:
                        cs = slice(0, 128)
                        pre = mmpool.tile([P, 128], F32, tag="mm", name="pre3")
                        pim = mmpool.tile([P, 128], F32, tag="mm", name="pim3")
                        nc.tensor.matmul(pre, mmcast(Ar), mmcast(S_re[:, cs]),
                                         start=True, stop=False)
                        nc.tensor.matmul(pre, mmcast(nAi), mmcast(S_im[:, cs]),
                                         start=False, stop=True)
                        nc.tensor.matmul(pim, mmcast(Ai), mmcast(S_re[:, cs]),
                                         start=True, stop=False)
                        nc.tensor.matmul(pim, mmcast(Ar), mmcast(S_im[:, cs]),
                                         start=False, stop=True)
                        nc.scalar.copy(T_re[:, cs], pre)
                        nc.vector.tensor_copy(T_im[:, cs], pim)
                        for pl_in, pl_out, eng in ((T_re, U_re, nc.scalar),
                                                   (T_im, U_im, nc.vector)):
                            pt = tppool.tile([P, P], F32, tag="tp", name="pt3")
                            nc.tensor.transpose(mmcast(pt),
                                                mmcast(pl_in[:, cs]),
                                                mmcast(ident))
                            cp = (eng.copy if eng is nc.scalar
                                  else eng.tensor_copy)
                            cp(u_major_view(pl_out, 0), pt)
                        for (ucs, vcs, br, bi, nbi) in (
                            (slice(0, 64), slice(0, 64), Br, Bi, nBi),
                            (slice(2048, 2112), slice(64, 128),
                             BrX, BiX, nBiX),
                        ):
                            pre = mmpool.tile([P, 64], F32, tag="mm",
                                              name="pre4")
                            pim = mmpool.tile([P, 64], F32, tag="mm",
                                              name="pim4")
                            nc.tensor.matmul(pre, mmcast(br),
                                             mmcast(U_re[:, ucs]),
                                             start=True, stop=False)
                            nc.tensor.matmul(pre, mmcast(nbi),
                                             mmcast(U_im[:, ucs]),
                                             start=False, stop=True)
                            nc.tensor.matmul(pim, mmcast(bi),
                                             mmcast(U_re[:, ucs]),
                                             start=True, stop=False)
                            nc.tensor.matmul(pim, mmcast(br),
                                             mmcast(U_im[:, ucs]),
                                             start=False, stop=True)
                            nc.scalar.copy(V_re[:, vcs], pre)
                            nc.vector.tensor_copy(V_im[:, vcs], pim)
                        for pl_in, pl_out, eng in ((V_re, S_re, nc.scalar),
                                                   (V_im, S_im, nc.vector)):
                            pt = tppool.tile([P, P], F32, tag="tp", name="pt4")
                            nc.tensor.transpose(mmcast(pt),
                                                mmcast(pl_in[:, cs]),
                                                mmcast(ident))
                            cp = (eng.copy if eng is nc.scalar
                                  else eng.tensor_copy)
                            cp(pl_out[:, cs], pt)
                        continue
                    # hi-MM: T = A @ S
                    for c in range(8):
                        cs = slice(512 * c, 512 * c + 512)
                        pre = mmpool.tile([P, 512], F32, tag="mm", name="pre")
                        pim = mmpool.tile([P, 512], F32, tag="mm", name="pim")
                        nc.tensor.matmul(pre, mmcast(Ar), mmcast(S_re[:, cs]),
                                         start=True, stop=False)
                        nc.tensor.matmul(pre, mmcast(nAi), mmcast(S_im[:, cs]),
                                         start=False, stop=True)
                        nc.tensor.matmul(pim, mmcast(Ai), mmcast(S_re[:, cs]),
                                         start=True, stop=False)
                        nc.tensor.matmul(pim, mmcast(Ar), mmcast(S_im[:, cs]),
                                         start=False, stop=True)
                        nc.scalar.copy(T_re[:, cs], pre)
                        nc.vector.tensor_copy(T_im[:, cs], pim)
                    if last and tlevel == 'hi':
                        dbg_src = [T_re, T_im]
                        break
                    # transpose T -> U (q6-split layout)
                    for g in range(32):
                        gs = slice(128 * g, 128 * g + 128)
                        for pl_in, pl_out, eng in ((T_re, U_re, nc.scalar),
                                                   (T_im, U_im, nc.vector)):
                            pt = tppool.tile([P, P], F32, tag="tp", name="pt")
                            nc.tensor.transpose(mmcast(pt), mmcast(pl_in[:, gs]),
                                                mmcast(ident))
                            cp = (eng.copy if eng is nc.scalar
                                  else eng.tensor_copy)
                            cp(u_major_view(pl_out, g), pt)
                    if last and tlevel == 'tr':
                        dbg_src = [U_re, U_im]
                        break
                    # lo-MM: V = blockdiag(B or X7 B) @ U
                    for c in range(8):
                        cs = slice(512 * c, 512 * c + 512)
                        br, bi, nbi = (Br, Bi, nBi) if c < 4 else (BrX, BiX,
                                                                   nBiX)
                        pre = mmpool.tile([P, 512], F32, tag="mm", name="pre2")
                        pim = mmpool.tile([P, 512], F32, tag="mm", name="pim2")
                        nc.tensor.matmul(pre, mmcast(br), mmcast(U_re[:, cs]),
                                         start=True, stop=False)
                        nc.tensor.matmul(pre, mmcast(nbi), mmcast(U_im[:, cs]),
                                         start=False, stop=True)
                        nc.tensor.matmul(pim, mmcast(bi), mmcast(U_re[:, cs]),
                                         start=True, stop=False)
                        nc.tensor.matmul(pim, mmcast(br), mmcast(U_im[:, cs]),
                                         start=False, stop=True)
                        # scatter-evac: U-col q6*2048+g*64+h' -> V-col
                        # g*128+q6*64+h', so V blocks are contiguous h = (q6,h')
                        q6c = c // 4
                        gb = 8 * (c % 4)

                        def v_scat(plane):
                            v = plane.rearrange("p (g s h) -> p g s h",
                                                g=32, s=2)
                            return v[:, gb:gb + 8, q6c:q6c + 1, :]
                        nc.scalar.copy(v_scat(V_re), pre)
                        nc.vector.tensor_copy(v_scat(V_im), pim)
                    if last and tlevel == 'lo':
                        dbg_src = [V_re, V_im]
                        break
                    # transpose back V -> S
                    for g in range(32):
                        gs = slice(128 * g, 128 * g + 128)
                        for pl_in, pl_out, eng in ((V_re, S_re, nc.scalar),
                                                   (V_im, S_im, nc.vector)):
                            pt = tppool.tile([P, P], F32, tag="tp", name="ptb")
                            nc.tensor.transpose(mmcast(pt), mmcast(pl_in[:, gs]),
                                                mmcast(ident))
                            cp = (eng.copy if eng is nc.scalar
                                  else eng.tensor_copy)
                            cp(pl_out[:, gs], pt)

                if fast3:
                    # broadcast sample-0 state to all 128 sample slots
                    for pl, stg in ((S_re, T_re), (S_im, T_im)):
                        nc.vector.tensor_copy(stg[:, 0:32], pl[:, 0:32])
                        bc = stg[:, 0:32].unsqueeze(1).broadcast_to(
                            (P, 128, 32))
                        nc.vector.tensor_copy(
                            pl.rearrange("p (b u) -> p b u", u=32), bc)

                if trunc is not None and tlevel != 'full':
                    continue
                # ---------- measurement ----------
                p_t, sq_t = T_re, T_im
                nc.scalar.activation(p_t, S_re, AF.Square)
                nc.scalar.activation(sq_t, S_im, AF.Square)
                nc.vector.tensor_add(p_t, p_t, sq_t)

                # lo-signed partial sums r_pat[h, b] for distinct u-patterns.
                # u bit positions (q7..q11) -> wirepos 0..4.
                LO_PATS = {0: (), 1: (0,), 2: (0, 1), 3: (2,), 4: (2, 3),
                           5: (4,)}
                r_pats = {}
                for pid, bits in LO_PATS.items():
                    # reduce over u keeping `bits`; then fold signs
                    if not bits:
                        r = small.tile([P, P], F32, tag=f"rpat{pid}",
                                       name=f"rp{pid}")
                        nc.vector.reduce_sum(
                            r, p_t.rearrange("p (b u) -> p b u", u=32),
                            axis=AX.X)
                        r_pats[pid] = r
                        continue
                    a0, b0 = bits[0], bits[-1]
                    runw = 1 << (b0 - a0 + 1)
                    o_sz = 1 << a0
                    i_sz = 32 // (o_sz * runw)
                    w4 = scratch.tile([P, P * runw], F32, tag="w4",
                                      name=f"w4_{pid}")
                    if o_sz == 1 and i_sz > 1:
                        vv = p_t.rearrange("p (b t i) -> p b t i", b=P, t=runw)
                        nc.vector.reduce_sum(w4, vv, axis=AX.X)
                    elif i_sz == 1 and o_sz > 1:
                        vv = p_t.rearrange("p (b o t) -> p b t o", b=P, t=runw)
                        nc.vector.reduce_sum(w4, vv, axis=AX.X)
                    else:
                        vv = p_t.rearrange("p (b o t i) -> p b t o i",
                                           b=P, o=o_sz, t=runw)
                        nc.vector.reduce_sum(w4, vv, axis=AX.XY)
                    # fold over run bits
                    src, width = w4, runw
                    while width > 1:
                        width //= 2
                        dst = (scratch.tile([P, P * width], F32, tag="fold2",
                                            name="fold2")
                               if width > 1 else
                               small.tile([P, P], F32, tag=f"rpat{pid}",
                                          name=f"rpf{pid}"))
                        s2 = src.rearrange("p (b t) -> p b t", t=2 * width)
                        nc.vector.tensor_sub(
                            dst.rearrange("p (b t) -> p b t", t=width),
                            s2[:, :, 0:width], s2[:, :, width:2 * width])
                        src = dst
                    r_pats[pid] = src

                # transpose the needed r matrices -> [b, h]
                # hi qubits use pattern 0; lo qubit q uses its own pattern.
                rT = {}
                for pid, r in r_pats.items():
                    pt = tppool.tile([P, P], F32, tag="tp", name="ptr")
                    nc.tensor.transpose(pt, r, ident)
                    rt = small.tile([P, P], F32, tag=f"rT{pid}",
                                    name=f"rT{pid}")
                    nc.scalar.copy(rt, pt)
                    rT[pid] = rt

                z_e = small.tile([P, N], F32, tag=f"z_e{enc_i}",
                                 name=f"z_e{enc_i}")
                # per-qubit signed reduce over h (free dim of rT)
                T_HI = {0: [0], 1: [1], 2: [1, 2], 3: [3], 4: [3, 4], 5: [5],
                        6: [5, 6]}
                T_LO = {7: [7], 8: [7, 8], 9: [9], 10: [9, 10], 11: [11]}
                for q in range(N):
                    if q <= 6:
                        src_m = rT[0]
                        bits = sorted(HI_ORDER.index(b2) for b2 in T_HI[q])
                        nb = 7
                    else:
                        pid = {7: 1, 8: 2, 9: 3, 10: 4, 11: 5}[q]
                        src_m = rT[pid]
                        bits = []
                        nb = 7
                    if not bits:
                        nc.vector.reduce_sum(z_e[:, q:q + 1], src_m, axis=AX.X)
                        continue
                    # reduce over h keeping `bits`; rest grouped into
                    # contiguous runs (always <= 2 runs here)
                    rest = [i for i in range(nb) if i not in bits]
                    runs = []
                    for i in rest:
                        if runs and runs[-1][-1] == i - 1:
                            runs[-1].append(i)
                        else:
                            runs.append([i])
                    assert len(runs) <= 2, (bits, runs)
                    names = [f"h{i}" for i in range(nb)]
                    pat = "p ({}) -> p {} {}".format(
                        " ".join(names),
                        " ".join(names[i] for i in bits),
                        " ".join("(" + " ".join(names[j] for j in run) + ")"
                                 for run in runs))
                    vv = src_m.rearrange(pat, **{n: 2 for n in names[:-1]})
                    kw = 1 << len(bits)
                    wq = scratch.tile([P, kw], F32, tag="wq", name="wq")
                    nc.vector.reduce_sum(
                        wq, vv, axis=AX.X if len(runs) == 1 else AX.XY)
                    srcf, width = wq, kw
                    while width > 1:
                        width //= 2
                        dstf = (z_e[:, q:q + 1] if width == 1 else
                                scratch.tile([P, width], F32, tag="foldq",
                                             name="foldq"))
                        nc.vector.tensor_sub(dstf, srcf[:, 0:width],
                                             srcf[:, width:2 * width])
                        srcf = dstf
                        # width already halved
                nc.vector.scalar_tensor_tensor(
                    zacc, z_e, wts[:, enc_i:enc_i + 1], zacc,
                    op0=OP.mult, op1=OP.add)

            if trunc is not None:
                nc.sync.dma_start(out=dbg[:, 0:DIM], in_=dbg_src[0])
                nc.sync.dma_start(out=dbg[:, DIM:2 * DIM], in_=dbg_src[1])
            nc.sync.dma_start(out=out, in_=zacc)

    nc.finalize()
    return nc


STAGE = 2

_NC_CACHE = None
LAST_RESULTS = None  # BassKernelResults of the most recent run (for profiling)


def kernel(features: np.ndarray, theta: np.ndarray, alpha: np.ndarray) -> np.ndarray:
    global _NC_CACHE, LAST_RESULTS
    if _NC_CACHE is None:
        _NC_CACHE = build_nc_stage2() if STAGE == 2 else build_nc()
    nc = _NC_CACHE

    features = np.ascontiguousarray(features, dtype=np.float32)
    theta = np.ascontiguousarray(theta, dtype=np.float32)
    alpha = np.ascontiguousarray(alpha, dtype=np.float32)

    B = features.shape[0]
    assert B == B_CORE * N_CORES, B
    in_maps = [
        {"features": features[i * B_CORE:(i + 1) * B_CORE], "theta": theta,
         "alpha": alpha}
        for i in range(N_CORES)
    ]
    res = run_bass_kernel_spmd(nc, in_maps, core_ids=list(range(N_CORES)))
    LAST_RESULTS = res
    return np.concatenate([r["out"] for r in res.results], axis=0)


if __name__ == "__main__":
    feats = np.random.rand(1024, 12).astype(np.float32)
    th = (0.01 * np.random.randn(4, 12, 3)).astype(np.float32)
    al = np.zeros(4, np.float32)
    y = kernel(feats, th, al)
    print(y.shape, y.dtype, y[:2])
